# revision 2
# baseline (speedup 1.0000x reference)
import sys

sys.path.insert(0, "/opt/trn_rl_repo")

from contextlib import ExitStack

import numpy as np

import concourse.bass as bass
import concourse.bacc as bacc
import concourse.mybir as mybir
import concourse.tile as tile
from concourse.bass_utils import run_bass_kernel_spmd
from concourse.masks import make_identity

N = 50000
E = 1600000
IN = 128
H = 4
C = 16
HC = H * C
P = 128
NCORES = 8
NPC = N // NCORES            # 6250 nodes per core
NT = (NPC + P - 1) // P      # 49 tiles per core
NPC_PAD = NT * P             # 6272
NXT = (N + P - 1) // P       # 391 x tiles
NX_PAD = NXT * P             # 50048

_cache = {}


def build_program(D):
    f32 = mybir.dt.float32
    i32 = mybir.dt.int32
    X = mybir.AxisListType.X
    mult = mybir.AluOpType.mult
    add = mybir.AluOpType.add
    sub = mybir.AluOpType.subtract
    amax = mybir.AluOpType.max

    nc = bacc.Bacc(None, target_bir_lowering=False, debug=True)
    x_ext = nc.declare_dram_parameter("x", [NX_PAD, IN], f32, isOutput=False)
    wt_ext = nc.declare_dram_parameter("wt", [IN, HC], f32, isOutput=False)
    asrc_ext = nc.declare_dram_parameter("attsrc", [P, HC], f32, isOutput=False)
    adst_ext = nc.declare_dram_parameter("attdst", [P, HC], f32, isOutput=False)
    bias_ext = nc.declare_dram_parameter("bias", [P, HC], f32, isOutput=False)
    idx_ext = nc.declare_dram_parameter("idx", [NPC_PAD, D], i32, isOutput=False)
    own_ext = nc.declare_dram_parameter("own", [NPC_PAD, 1], i32, isOutput=False)
    mask_ext = nc.declare_dram_parameter("mask", [NPC_PAD, D], f32, isOutput=False)
    out_ext = nc.declare_dram_parameter("out", [NPC_PAD, HC], f32, isOutput=True)
    xp_dram = nc.dram_tensor("xp", [NX_PAD, HC], f32)

    with tile.TileContext(nc) as tc, ExitStack() as ctx:
        singles = ctx.enter_context(tc.tile_pool(name="singles", bufs=1))
        xbufs = ctx.enter_context(tc.tile_pool(name="xbufs", bufs=4))
        psums = ctx.enter_context(tc.tile_pool(name="psums", bufs=4, space="PSUM"))
        gath = ctx.enter_context(tc.tile_pool(name="gath", bufs=2))
        small = ctx.enter_context(tc.tile_pool(name="small", bufs=4))

        ident = singles.tile([P, P], f32)
        make_identity(nc, ident[:])
        wt_sb = singles.tile([IN, HC], f32)
        nc.sync.dma_start(out=wt_sb[:], in_=wt_ext[:])
        attsrc_sb = singles.tile([P, HC], f32)
        nc.sync.dma_start(out=attsrc_sb[:], in_=asrc_ext[:])
        attdst_sb = singles.tile([P, HC], f32)
        nc.sync.dma_start(out=attdst_sb[:], in_=adst_ext[:])
        bias_sb = singles.tile([P, HC], f32)
        nc.sync.dma_start(out=bias_sb[:], in_=bias_ext[:])

        # Phase 1: xp = x @ W.T  (xp[n, h*16+c])
        for t in range(NXT):
            r0 = t * P
            xt = xbufs.tile([P, IN], f32)
            nc.sync.dma_start(out=xt[:], in_=x_ext[r0:r0 + P, :])
            xT_ps = psums.tile([P, P], f32, space="PSUM")
            nc.tensor.transpose(out=xT_ps[:], in_=xt[:], identity=ident[:])
            xT_sb = xbufs.tile([P, P], f32)
            nc.scalar.copy(out=xT_sb[:], in_=xT_ps[:])
            xp_ps = psums.tile([P, HC], f32, space="PSUM")
            nc.tensor.matmul(out=xp_ps[:], lhsT=xT_sb[:], rhs=wt_sb[:],
                             start=True, stop=True)
            xp_sb = xbufs.tile([P, HC], f32)
            nc.vector.tensor_copy(out=xp_sb[:], in_=xp_ps[:])
            nc.sync.dma_start(out=xp_dram[r0:r0 + P, :], in_=xp_sb[:])

        # broadcast views of the replicated params (d-major layouts)
        attsrc_b = attsrc_sb[:].rearrange(
            "p (one h c) -> p one h c", one=1, h=H, c=C).to_broadcast([P, D, H, C])

        # Phase 2: per node-tile gather + softmax + aggregate
        for t in range(NT):
            r0 = t * P
            idx_sb = gath.tile([P, D], i32)
            nc.sync.dma_start(out=idx_sb[:], in_=idx_ext[r0:r0 + P, :])
            own_sb = gath.tile([P, 1], i32)
            nc.sync.dma_start(out=own_sb[:], in_=own_ext[r0:r0 + P, :])
            mask_sb = gath.tile([P, D], f32)
            nc.sync.dma_start(out=mask_sb[:], in_=mask_ext[r0:r0 + P, :])

            gown = gath.tile([P, HC], f32)
            nc.gpsimd.indirect_dma_start(
                out=gown[:], out_offset=None, in_=xp_dram[:],
                in_offset=bass.IndirectOffsetOnAxis(ap=own_sb[:, :1], axis=0))

            xpg = gath.tile([P, D * H * C], f32)
            xpg3 = xpg[:].rearrange("p (d hc) -> p d hc", d=D, hc=HC)
            xpg4 = xpg[:].rearrange("p (d h c) -> p d h c", d=D, h=H, c=C)
            for d in range(D):
                nc.gpsimd.indirect_dma_start(
                    out=xpg3[:, d, :], out_offset=None, in_=xp_dram[:],
                    in_offset=bass.IndirectOffsetOnAxis(ap=idx_sb[:, d:d + 1], axis=0))

            # a_dst for the tile's own nodes: [P, H]
            tmp64 = small.tile([P, HC], f32)
            nc.vector.tensor_tensor(out=tmp64[:], in0=gown[:], in1=attdst_sb[:],
                                    op=mult)
            adst = small.tile([P, H], f32)
            nc.vector.tensor_reduce(
                out=adst[:], in_=tmp64[:].rearrange("p (h c) -> p h c", h=H, c=C),
                axis=X, op=add)

            # a_src per edge slot: [P, D, H]
            prod = gath.tile([P, D * H * C], f32)
            prod4 = prod[:].rearrange("p (d h c) -> p d h c", d=D, h=H, c=C)
            nc.vector.tensor_tensor(out=prod4, in0=xpg4, in1=attsrc_b, op=mult)
            e = gath.tile([P, D * H], f32)
            e3 = e[:].rearrange("p (d h) -> p d h", d=D, h=H)
            nc.vector.tensor_reduce(out=e3, in_=prod4, axis=X, op=add)

            # e = leaky_relu(a_src + a_dst) + mask
            adst_b = adst[:].rearrange("p (one h) -> p one h", one=1, h=H) \
                .to_broadcast([P, D, H])
            nc.vector.tensor_tensor(out=e3, in0=e3, in1=adst_b, op=add)
            et = gath.tile([P, D * H], f32)
            et3 = et[:].rearrange("p (d h) -> p d h", d=D, h=H)
            nc.scalar.mul(out=et3, in_=e3, mul=0.2)
            nc.vector.tensor_tensor(out=e3, in0=e3, in1=et3, op=amax)
            mask_b = mask_sb[:].rearrange("p (d one) -> p d one", d=D, one=1) \
                .to_broadcast([P, D, H])
            nc.vector.tensor_tensor(out=e3, in0=e3, in1=mask_b, op=add)

            # segment softmax over d (dense)
            m = small.tile([P, H], f32)
            nc.vector.tensor_reduce(
                out=m[:], in_=e[:].rearrange("p (d h) -> p h d", d=D, h=H),
                axis=X, op=amax)
            m_b = m[:].rearrange("p (one h) -> p one h", one=1, h=H) \
                .to_broadcast([P, D, H])
            nc.vector.tensor_tensor(out=e3, in0=e3, in1=m_b, op=sub)
            nc.scalar.activation(out=e3, in_=e3,
                                 func=mybir.ActivationFunctionType.Exp)
            s = small.tile([P, H], f32)
            nc.vector.tensor_reduce(
                out=s[:], in_=e[:].rearrange("p (d h) -> p h d", d=D, h=H),
                axis=X, op=add)
            sinv = small.tile([P, H], f32)
            nc.vector.reciprocal(out=sinv[:], in_=s[:])
            sinv_b = sinv[:].rearrange("p (one h) -> p one h", one=1, h=H) \
                .to_broadcast([P, D, H])
            nc.vector.tensor_tensor(out=e3, in0=e3, in1=sinv_b, op=mult)

            # weighted aggregate: out[p, h, c] = sum_d alpha[p,d,h] * xpg[p,d,h,c]
            alpha_b = e[:].rearrange("p (d h one) -> p d h one", d=D, h=H, one=1) \
                .to_broadcast([P, D, H, C])
            nc.vector.tensor_tensor(out=xpg4, in0=xpg4, in1=alpha_b, op=mult)
            acc = small.tile([P, HC], f32)
            nc.vector.tensor_reduce(
                out=acc[:],
                in_=xpg[:].rearrange("p (d h c) -> p h c d", d=D, h=H, c=C),
                axis=X, op=add)
            outsb = small.tile([P, HC], f32)
            nc.vector.tensor_tensor(out=outsb[:], in0=acc[:], in1=bias_sb[:], op=add)
            nc.sync.dma_start(out=out_ext[r0:r0 + P, :], in_=outsb[:])

    nc.compile()
    return nc


def _preprocess(edge_index):
    src = edge_index[0].astype(np.int32)
    dst = edge_index[1].astype(np.int32)
    counts = np.bincount(dst, minlength=N)
    D = int(counts.max())
    order = np.argsort(dst, kind="stable")
    dst_s = dst[order]
    src_s = src[order]
    starts = np.zeros(N + 1, np.int64)
    np.cumsum(counts, out=starts[1:])
    slot = np.arange(E, dtype=np.int64) - starts[dst_s]
    idx_all = np.zeros((N, D), np.int32)
    mask_all = np.full((N, D), -1e30, np.float32)
    idx_all[dst_s, slot] = src_s
    mask_all[dst_s, slot] = 0.0
    return D, idx_all, mask_all


def kernel(x, edge_index, W, att_src, att_dst, bias):
    x = np.asarray(x, np.float32)
    edge_index = np.asarray(edge_index)
    W = np.asarray(W, np.float32)
    att_src = np.asarray(att_src, np.float32)
    att_dst = np.asarray(att_dst, np.float32)
    bias = np.asarray(bias, np.float32)

    D, idx_all, mask_all = _preprocess(edge_index)
    if D not in _cache:
        _cache[D] = build_program(D)
    nc = _cache[D]

    x_pad = np.zeros((NX_PAD, IN), np.float32)
    x_pad[:N] = x
    wt = np.ascontiguousarray(W.T)
    attsrc_rep = np.tile(att_src.reshape(1, HC), (P, 1)).astype(np.float32)
    attdst_rep = np.tile(att_dst.reshape(1, HC), (P, 1)).astype(np.float32)
    bias_rep = np.tile(bias.reshape(1, HC), (P, 1)).astype(np.float32)

    in_maps = []
    for c in range(NCORES):
        lo, hi = c * NPC, (c + 1) * NPC
        idx_c = np.zeros((NPC_PAD, D), np.int32)
        idx_c[:NPC] = idx_all[lo:hi]
        mask_c = np.full((NPC_PAD, D), -1e30, np.float32)
        mask_c[:NPC] = mask_all[lo:hi]
        own_c = np.zeros((NPC_PAD, 1), np.int32)
        own_c[:NPC, 0] = np.arange(lo, hi, dtype=np.int32)
        in_maps.append({
            "x": x_pad, "wt": wt, "attsrc": attsrc_rep, "attdst": attdst_rep,
            "bias": bias_rep, "idx": idx_c, "own": own_c, "mask": mask_c,
        })

    r = run_bass_kernel_spmd(nc, in_maps, list(range(NCORES)))
    globals()["LAST_RESULTS"] = r
    res = r.results
    out = np.concatenate([np.asarray(res[c]["out"])[:NPC] for c in range(NCORES)],
                         axis=0)
    return out.astype(np.float32)



# revision 6
# speedup vs baseline: 3.4457x; 3.4457x over previous
import sys

sys.path.insert(0, "/opt/trn_rl_repo")

from contextlib import ExitStack

import numpy as np
import ml_dtypes

import concourse.bass as bass
import concourse.bacc as bacc
import concourse.mybir as mybir
import concourse.tile as tile
from concourse.bass_utils import run_bass_kernel_spmd
from concourse.masks import make_identity

N = 50000
E = 1600000
IN = 128
H = 4
C = 16
HC = H * C
ROW = 128          # bf16 elems per node row: xp(64) + a_src(4) + a_dst(4) + pad
P = 128
NCORES = 8
NT = 49                      # node tiles per core
CHUNK = N // NCORES // P + 1  # not used directly; NT covers 6272 >= 6250
NPC = NT * P                 # 6272 padded nodes per core
NX_PAD = NCORES * NPC        # 50176 >= 50000 padded node space
NPAIR = NX_PAD // 2          # 25088 pair rows (int16-addressable)
SENT = NX_PAD - 2            # sentinel node ids: SENT, SENT+1 (pair NPAIR-1)
XB = 4                       # x tiles per phase-1 chunk
NXC = NX_PAD // (XB * P)     # 98 phase-1 chunks

_cache = {}


def build_program(dlist):
    f32 = mybir.dt.float32
    bf16 = mybir.dt.bfloat16
    i16 = mybir.dt.int16
    X = mybir.AxisListType.X
    mult = mybir.AluOpType.mult
    add = mybir.AluOpType.add
    sub = mybir.AluOpType.subtract
    amax = mybir.AluOpType.max

    nslot = sum(d + 1 for d in dlist)  # per-partition slots incl own block

    nc = bacc.Bacc(None, target_bir_lowering=False, debug=True)
    x_ext = nc.declare_dram_parameter("x", [NX_PAD, IN], f32, isOutput=False)
    wt_ext = nc.declare_dram_parameter("wt", [IN, ROW], f32, isOutput=False)
    sent_ext = nc.declare_dram_parameter("sent", [2, ROW], f32, isOutput=False)
    bias_ext = nc.declare_dram_parameter("bias", [P, HC], f32, isOutput=False)
    idx_ext = nc.declare_dram_parameter("idx", [P, nslot * P // 16], i16,
                                        isOutput=False)
    hbit_ext = nc.declare_dram_parameter("hbit", [P, nslot], bf16, isOutput=False)
    out_ext = nc.declare_dram_parameter("out", [NPC, HC], f32, isOutput=True)
    xp_dram = nc.dram_tensor("xp", [NPAIR, 2 * ROW], bf16)
    xp_flat = xp_dram[:].rearrange("r (two c) -> (r two) c", two=2, c=ROW)

    with tile.TileContext(nc) as tc, ExitStack() as ctx:
        singles = ctx.enter_context(tc.tile_pool(name="singles", bufs=1))
        xbufs = ctx.enter_context(tc.tile_pool(name="xbufs", bufs=3))
        psums = ctx.enter_context(tc.tile_pool(name="psums", bufs=4, space="PSUM"))
        gath = ctx.enter_context(tc.tile_pool(name="gath", bufs=2))
        work = ctx.enter_context(tc.tile_pool(name="work", bufs=2))
        small = ctx.enter_context(tc.tile_pool(name="small", bufs=4))

        ident = singles.tile([P, P], f32)
        make_identity(nc, ident[:])
        wt_sb = singles.tile([IN, ROW], f32)
        nc.sync.dma_start(out=wt_sb[:], in_=wt_ext[:])
        bias_sb = singles.tile([P, HC], f32)
        nc.sync.dma_start(out=bias_sb[:], in_=bias_ext[:])
        idx_all = singles.tile([P, nslot * P // 16], i16)
        nc.sync.dma_start(out=idx_all[:], in_=idx_ext[:])
        hbit_all = singles.tile([P, nslot], bf16)
        nc.sync.dma_start(out=hbit_all[:], in_=hbit_ext[:])

        # Phase 1: xp_aug = x @ [W.T | vsrc | vdst | 0...]  stored bf16
        for ch in range(NXC):
            r0 = ch * XB * P
            xt = xbufs.tile([P, XB * IN], f32)
            nc.sync.dma_start(
                out=xt[:].rearrange("p (b c) -> p b c", b=XB, c=IN),
                in_=x_ext[r0:r0 + XB * P, :].rearrange(
                    "(b p) c -> p b c", b=XB, p=P))
            xpo = xbufs.tile([P, XB * ROW], bf16)
            for b in range(XB):
                xT_ps = psums.tile([P, P], f32, space="PSUM")
                nc.tensor.transpose(out=xT_ps[:], in_=xt[:, b * IN:(b + 1) * IN],
                                    identity=ident[:])
                xT_sb = xbufs.tile([P, P], f32)
                nc.scalar.copy(out=xT_sb[:], in_=xT_ps[:])
                xp_ps = psums.tile([P, ROW], f32, space="PSUM")
                nc.tensor.matmul(out=xp_ps[:], lhsT=xT_sb[:], rhs=wt_sb[:],
                                 start=True, stop=True)
                nc.vector.tensor_copy(out=xpo[:, b * ROW:(b + 1) * ROW],
                                      in_=xp_ps[:])
            nc.sync.dma_start(
                out=xp_flat[r0:r0 + XB * P, :].rearrange(
                    "(b p) c -> p b c", b=XB, p=P),
                in_=xpo[:].rearrange("p (b c) -> p b c", b=XB, c=ROW))

        # overwrite sentinel rows (a_src = -1e30, xp = 0)
        sent_sb = singles.tile([2, ROW], f32)
        nc.sync.dma_start(out=sent_sb[:], in_=sent_ext[:])
        sent_bf = singles.tile([2, ROW], bf16)
        nc.vector.tensor_copy(out=sent_bf[:], in_=sent_sb[:])
        nc.sync.dma_start(out=xp_flat[SENT:SENT + 2, :], in_=sent_bf[:])

        # Phase 2
        soff = 0  # slot offset (per-partition) into idx/hbit
        for t in range(NT):
            D = dlist[t]
            NB = D + 1                      # neighbor blocks + own block last
            NI = NB * P

            g = gath.tile([P, NB * 2 * ROW], bf16)
            g4 = g[:].rearrange("p (nb two c) -> p nb two c", nb=NB, two=2, c=ROW)
            nc.gpsimd.dma_gather(
                g[:].rearrange("p (nb e) -> p nb e", nb=NB, e=2 * ROW),
                xp_dram[:],
                idx_all[:, soff * P // 16:(soff + NB) * P // 16],
                NI, NI, 2 * ROW, single_packet=False)

            # select half: sel = left + h*(right-left), cols 0:72
            selw = 72
            diff = work.tile([P, NB * selw], bf16)
            diff3 = diff[:].rearrange("p (nb c) -> p nb c", nb=NB, c=selw)
            nc.vector.tensor_tensor(out=diff3, in0=g4[:, :, 1, 0:selw],
                                    in1=g4[:, :, 0, 0:selw], op=sub)
            hb = hbit_all[:, soff:soff + NB].rearrange(
                "p (nb one) -> p nb one", nb=NB, one=1).to_broadcast([P, NB, selw])
            nc.vector.tensor_tensor(out=diff3, in0=diff3, in1=hb, op=mult)
            sel = work.tile([P, NB * selw], bf16)
            sel3 = sel[:].rearrange("p (nb c) -> p nb c", nb=NB, c=selw)
            nc.vector.tensor_tensor(out=sel3, in0=diff3, in1=g4[:, :, 0, 0:selw],
                                    op=add)
            sel4h = sel[:].rearrange("p (nb c) -> p nb c", nb=NB, c=selw)

            # scores: e[p,d,h] = leaky(asrc_nb[p,d,h] + adst_own[p,h])
            adst = small.tile([P, H], bf16)
            nc.vector.tensor_copy(out=adst[:], in_=sel4h[:, D, 68:72])
            e = work.tile([P, D * H], f32)
            e3 = e[:].rearrange("p (d h) -> p d h", d=D, h=H)
            adst_b = adst[:].rearrange("p (one h) -> p one h", one=1, h=H) \
                .to_broadcast([P, D, H])
            nc.vector.tensor_tensor(out=e3, in0=sel4h[:, 0:D, 64:68],
                                    in1=adst_b, op=add)
            et = work.tile([P, D * H], f32)
            et3 = et[:].rearrange("p (d h) -> p d h", d=D, h=H)
            nc.scalar.mul(out=et3, in_=e3, mul=0.2)
            nc.vector.tensor_tensor(out=e3, in0=e3, in1=et3, op=amax)

            # softmax over d
            m = small.tile([P, H], f32)
            nc.vector.tensor_reduce(
                out=m[:], in_=e[:].rearrange("p (d h) -> p h d", d=D, h=H),
                axis=X, op=amax)
            m_b = m[:].rearrange("p (one h) -> p one h", one=1, h=H) \
                .to_broadcast([P, D, H])
            nc.vector.tensor_tensor(out=e3, in0=e3, in1=m_b, op=sub)
            nc.scalar.activation(out=e3, in_=e3,
                                 func=mybir.ActivationFunctionType.Exp)
            s = small.tile([P, H], f32)
            nc.vector.tensor_reduce(
                out=s[:], in_=e[:].rearrange("p (d h) -> p h d", d=D, h=H),
                axis=X, op=add)
            sinv = small.tile([P, H], f32)
            nc.vector.reciprocal(out=sinv[:], in_=s[:])
            sinv_b = sinv[:].rearrange("p (one h) -> p one h", one=1, h=H) \
                .to_broadcast([P, D, H])
            nc.vector.tensor_tensor(out=e3, in0=e3, in1=sinv_b, op=mult)

            # weighted aggregate over d
            wm = work.tile([P, D * HC], f32)
            wm4 = wm[:].rearrange("p (d h c) -> p d h c", d=D, h=H, c=C)
            alpha_b = e[:].rearrange("p (d h one) -> p d h one", d=D, h=H, one=1) \
                .to_broadcast([P, D, H, C])
            selx = sel[:].rearrange("p (nb c) -> p nb c", nb=NB, c=selw)[:, 0:D, 0:64]
            nc.vector.tensor_tensor(
                out=wm4, in0=selx.rearrange("p d (h c) -> p d h c", h=H, c=C),
                in1=alpha_b, op=mult)
            acc = small.tile([P, HC], f32)
            nc.vector.tensor_reduce(
                out=acc[:],
                in_=wm[:].rearrange("p (d h c) -> p h c d", d=D, h=H, c=C),
                axis=X, op=add)
            outsb = small.tile([P, HC], f32)
            nc.vector.tensor_tensor(out=outsb[:], in0=acc[:], in1=bias_sb[:],
                                    op=add)
            nc.sync.dma_start(out=out_ext[t * P:(t + 1) * P, :], in_=outsb[:])

            soff += NB

    nc.compile()
    return nc


def _preprocess(edge_index):
    src = edge_index[0].astype(np.int64)
    dst = edge_index[1].astype(np.int64)
    deg = np.bincount(dst, minlength=N)

    # degree-sorted permutation: newid order = nodes sorted by degree desc
    order = np.argsort(-deg, kind="stable")       # old ids in new order
    # shared-D chunks of 1024 new-ids -> tile slot s on every core
    newid = np.empty(N + NX_PAD - N, np.int64)
    newid_of_old = np.empty(N, np.int64)
    newid_of_old[order] = np.arange(N)
    # chunk s covers new-ids [s*1024, (s+1)*1024); core c gets [c*128, (c+1)*128)
    deg_sorted = deg[order]
    dlist = []
    for s in range(NT):
        lo = s * NCORES * P
        hi = min(lo + NCORES * P, N)
        d = int(deg_sorted[lo:hi].max()) if lo < N else 0
        dlist.append(max(d, 1))

    # per-edge: slot assignment within (tile, partition)
    e_new_dst = newid_of_old[dst]
    e_new_src = newid_of_old[src]
    sort_o = np.argsort(e_new_dst, kind="stable")
    sd = e_new_dst[sort_o]
    ss = e_new_src[sort_o]
    starts = np.zeros(N + 1, np.int64)
    np.cumsum(np.bincount(sd, minlength=N), out=starts[1:])
    slot = np.arange(E, dtype=np.int64) - starts[sd]

    # new-id -> (core, tile, partition)
    chunk = sd // (NCORES * P)
    within = sd % (NCORES * P)
    core = within // P
    part = within % P

    # physical storage id of a new-id n (node rows in xp, padded per core):
    # store new-id n at physrow = core*NPC + tile*128 + part
    phys_dst = core * NPC + chunk * P + part
    phys_src_all = np.full(NX_PAD, SENT, np.int64)
    nid = np.arange(N)
    w_all = nid % (NCORES * P)
    phys_of_new = (w_all // P) * NPC + (nid // (NCORES * P)) * P + (w_all % P)
    phys_src = phys_of_new[ss]

    # build per-core idx16 (pair rows) and hbit arrays
    nslot = sum(d + 1 for d in dlist)
    idx16 = np.full((NCORES, P, nslot), NPAIR - 1, np.int16)
    hbit = np.zeros((NCORES, P, nslot), np.float32)
    # slot offsets per tile
    soffs = np.zeros(NT, np.int64)
    acc = 0
    for t in range(NT):
        soffs[t] = acc
        acc += dlist[t] + 1
    e_soff = soffs[chunk] + slot
    idx16[core, part, e_soff] = (phys_src >> 1).astype(np.int16)
    hbit[core, part, e_soff] = (phys_src & 1).astype(np.float32)
    # own blocks: slot soffs[t] + dlist[t]
    own_new = np.arange(N)
    own_chunk = own_new // (NCORES * P)
    own_within = own_new % (NCORES * P)
    own_core = own_within // P
    own_part = own_within % P
    own_phys = phys_of_new[own_new]
    own_soff = soffs[own_chunk] + np.asarray(dlist)[own_chunk]
    idx16[own_core, own_part, own_soff] = (own_phys >> 1).astype(np.int16)
    hbit[own_core, own_part, own_soff] = (own_phys & 1).astype(np.float32)

    # x permuted into physical storage order
    x_perm_rows = np.full(NX_PAD, -1, np.int64)   # phys row -> old node id
    x_perm_rows[phys_of_new] = order[np.arange(N)]

    return dlist, idx16, hbit, x_perm_rows, phys_of_new, order


def _wrap_idx(idx_core, nslot):
    # flat slot i (= slotblock*128 + p) consumed from wrapped[i%16, i//16],
    # partitions replicated mod 16
    flat = idx_core.T.reshape(-1)                  # i = sblock*128 + p
    ncol = nslot * P // 16
    w16 = flat.reshape(ncol, 16).T                 # [16, ncol]
    return np.tile(w16, (8, 1)).astype(np.int16)   # [128, ncol]


def kernel(x, edge_index, W, att_src, att_dst, bias):
    x = np.asarray(x, np.float32)
    edge_index = np.asarray(edge_index)
    W = np.asarray(W, np.float32)
    att_src = np.asarray(att_src, np.float32)
    att_dst = np.asarray(att_dst, np.float32)
    bias = np.asarray(bias, np.float32)

    dlist, idx16, hbit, x_perm_rows, phys_of_new, order = _preprocess(edge_index)
    key = tuple(dlist)
    if key not in _cache:
        _cache[key] = build_program(list(dlist))
    nc = _cache[key]

    # augmented weights: [W.T | vsrc | vdst | zeros]  -> [IN, ROW]
    vsrc = (W.T.reshape(IN, H, C) * att_src[None]).sum(-1)   # [IN, H]
    vdst = (W.T.reshape(IN, H, C) * att_dst[None]).sum(-1)   # [IN, H]
    wt = np.zeros((IN, ROW), np.float32)
    wt[:, :HC] = W.T
    wt[:, HC:HC + H] = vsrc
    wt[:, HC + H:HC + 2 * H] = vdst

    x_pad = np.zeros((NX_PAD, IN), np.float32)
    valid = x_perm_rows >= 0
    x_pad[valid] = x[x_perm_rows[valid]]

    sent = np.zeros((2, ROW), np.float32)
    sent[:, HC:HC + H] = -1e30
    bias_rep = np.tile(bias.reshape(1, HC), (P, 1)).astype(np.float32)

    nslot = sum(d + 1 for d in dlist)
    in_maps = []
    for c in range(NCORES):
        in_maps.append({
            "x": x_pad, "wt": wt, "sent": sent, "bias": bias_rep,
            "idx": _wrap_idx(idx16[c], nslot),
            "hbit": hbit[c].astype(ml_dtypes.bfloat16),
        })

    r = run_bass_kernel_spmd(nc, in_maps, list(range(NCORES)))
    globals()["LAST_RESULTS"] = r
    res = r.results

    out = np.zeros((N, HC), np.float32)
    # result row (core, tile, part) = new-id tile*1024 + core*128 + part
    allrows = np.concatenate([np.asarray(res[c]["out"]) for c in range(NCORES)],
                             axis=0)  # [NCORES*NPC, HC]
    # phys row -> value; new-id n lives at phys_of_new[n]
    out[order[np.arange(N)]] = allrows[phys_of_new]
    return out.astype(np.float32)
